# revision 16
# baseline (speedup 1.0000x reference)
"""Multi-head attention (B=2, S=2048, D=1024, H=16) on 8 trn2 NeuronCores.

Sharding: batch (2) x head-groups (4 heads each, 4 groups) = 8 cores.
Each core computes Q/K/V projections for its 4 heads on its batch,
causal-masked softmax attention, and a partial output projection
(row-sharded w_o); the host sums the 4 partials per batch.

Layout strategy: the host stages transposed inputs (xT = x[b].T) so every
matmul contraction runs over the SBUF partition axis with no on-device
transposes. Attention scores are computed transposed (ST[k, q]) so that
P = exp(ST) serves directly as the PV matmul's moving operand.

v3 structure (269us -> 207us -> this):
- Scores run un-padded K=64 with two heads packed concurrently in the PE
  array via base_partition 0/64 (tile_position row groups).
- The PV stationary operand is [V_h | 64 columns of ones]: output rows
  0:64 are the attention output, rows 64:128 all accumulate the softmax
  denominator D (broadcast by the matmul for free). The normalizer
  1/D = Exp(-Ln(D)) runs on the Activation engine (both functions live in
  the natural_log_exp_and_others table - one load, no switches).
- There is no separate projection phase: Q/K/V projection matmul groups
  are "filler" tasks with emission deadlines, woven into the attention
  strip loop wherever the (ACT-bound) attention chain leaves PE slack,
  alongside the previous strip's output-projection tiles. Scores for
  k-tile N+1 are issued before PV of k-tile N so the PE FIFO never
  head-of-line blocks on exp. The PE therefore stays continuously busy
  and the HAM clock gate stays at 2.4 GHz.
- x^T input DMAs are issued on the gpsimd (software DGE) queue, strip-
  interleaved q0,k0,v0,q1,... so the first projection group can start
  ~5us in; weight DMAs ride the sync queue; y output DMAs share gpsimd.
- PSUM: tag-shared ring of 2x 4KiB slots (score pair tiles [128,2,512]f32,
  y-projection tiles, and 2KiB projection tiles all rotate through it)
  + pots [128,4,512]f32 (4 banks) = 8 banks exactly.
"""
import sys

sys.path.insert(0, "/opt/trn_rl_repo")

import numpy as np
import ml_dtypes

import concourse.bass as bass
import concourse.mybir as mybir
import concourse.tile as tile
from concourse.bass_utils import run_bass_kernel_spmd

B, S, D, H, DK = 2, 2048, 1024, 16, 64
NCORES = 8
HG = 4                # heads per core
DHG = HG * DK         # 256 head-dims per core
KT = D // 128         # 8 contraction tiles for the projections
ST128 = S // 128      # 16 128-row tiles of S
QS = 512              # q-strip width
NQS = S // QS         # 4 strips

f32 = mybir.dt.float32
bf16 = mybir.dt.bfloat16
EXP = mybir.ActivationFunctionType.Exp
LN = mybir.ActivationFunctionType.Ln


def _split_waits(nc, max_waits=1):
    """This walrus build rejects >1 SyncWait per instruction (and >0 on
    fp32-family matmuls, which lower through the 1-wait S3_LW struct).
    Hoist excess waits onto dedicated NOPs on the same engine queue."""
    n = 0
    for fn in nc.m.functions:
        for blk in fn.blocks:
            new = []
            for ins in blk.instructions:
                si = getattr(ins, "sync_info", None)
                if si is not None and si.on_wait:
                    limit = 0 if isinstance(ins, mybir.InstMatmult) else max_waits
                    if len(si.on_wait) > limit:
                        waits = list(si.on_wait)
                        hoist = waits if limit == 0 else waits[:-limit]
                        keep = [] if limit == 0 else waits[-limit:]
                        for w in hoist:
                            n += 1
                            new.append(
                                mybir.InstNoOp(
                                    name=f"I-waitfix-{n}",
                                    engine=ins.engine,
                                    bass_nofuse=True,
                                    sync_info=mybir.SyncInfo(
                                        on_wait=[w], on_update=[]
                                    ),
                                )
                            )
                        ins.sync_info = mybir.SyncInfo(
                            on_wait=keep, on_update=list(si.on_update)
                        )
                new.append(ins)
            blk.instructions[:] = new
    return n


def classify_mask(maskT):
    """Block-classify the transposed mask at 128x128 granularity.
    Returns (cls[i,j] in {0 empty,1 full,2 partial}, bias index map,
    list of multiplicative fp32 mask blocks for the partial ones)."""
    nb = S // 128
    cls = np.empty((nb, nb), dtype=np.int8)
    bidx = np.full((nb, nb), -1, dtype=np.int32)
    biases = []
    for i in range(nb):
        for j in range(nb):
            blk = maskT[i * 128 : (i + 1) * 128, j * 128 : (j + 1) * 128]
            if (blk != 0).all():
                cls[i, j] = 1
            elif (blk == 0).all():
                cls[i, j] = 0
            else:
                cls[i, j] = 2
                bidx[i, j] = len(biases)
                biases.append((blk != 0).astype(np.float32))
    return cls, bidx, biases


def strip_kts(cls, qs):
    sub_all = cls[:, 4 * qs : 4 * qs + 4]
    return [i for i in range(ST128) if sub_all[i].any()]


def build_program(cls, bidx, n_bias):
    nb_alloc = max(1, n_bias)
    nc = bass.Bass("TRN2", target_bir_lowering=False, debug=False,
                   num_devices=NCORES)
    xq_d = nc.dram_tensor("xqT", [D, S], bf16, kind="ExternalInput").ap()
    xk_d = nc.dram_tensor("xkT", [D, S], bf16, kind="ExternalInput").ap()
    xv_d = nc.dram_tensor("xvT", [D, S], bf16, kind="ExternalInput").ap()
    wq_d = nc.dram_tensor("wqT", [D, DHG], bf16, kind="ExternalInput").ap()
    wk_d = nc.dram_tensor("wkT", [D, DHG], bf16, kind="ExternalInput").ap()
    wv_d = nc.dram_tensor("wvT", [D, DHG], bf16, kind="ExternalInput").ap()
    wo_d = nc.dram_tensor("woT", [DHG, D], bf16, kind="ExternalInput").ap()
    bias_d = nc.dram_tensor("biasT", [nb_alloc, 128, HG, 128], bf16,
                            kind="ExternalInput").ap()
    y_d = nc.dram_tensor("y", [S, D], f32, kind="ExternalOutput").ap()

    with tile.TileContext(nc) as tc:
        with tc.tile_pool(name="persist", bufs=1) as pp, tc.tile_pool(
            name="xw", bufs=3
        ) as xw, tc.tile_pool(
            name="pb", bufs=4
        ) as pb, tc.tile_pool(
            name="dn", bufs=1
        ) as dn, tc.tile_pool(
            name="yb", bufs=2
        ) as ypool, tc.tile_pool(
            name="psS", bufs=2, space="PSUM"
        ) as psS, tc.tile_pool(
            name="psO", bufs=1, space="PSUM"
        ) as psO:
            # pair-major head layout: partitions 0:64 <-> head 2j, 64:128
            # <-> head 2j+1, for pair index j in {0,1}
            qt_sb = pp.tile([128, 2, S], bf16)            # Q^T
            kt_sb = pp.tile([128, 2, S], bf16)            # K^T
            v_sb = pp.tile([128, ST128, HG, 128], bf16)   # [V | ones]
            ot_sb = pp.tile([128, 2, S], bf16)            # attn out^T
            wo_sb = pp.tile([128, 2, D], bf16)
            bias_sb = pp.tile([128, nb_alloc, HG, 128], bf16)
            # denominator-broadcast columns of the PV stationary operand
            nc.gpsimd.memset(v_sb[:, :, :, DK:], 1.0)

            # ---- input DMAs: three queues so transfers run in parallel:
            # q on sync (hwdge), k on scalar (hwdge), v on gpsimd (swdge),
            # each queue loading its weight first then the x strips in
            # consumption order
            xts = {}
            wts = {}
            qeng = {"q": nc.sync, "k": nc.scalar, "v": nc.gpsimd}
            for which, x_d, w_d in (
                ("q", xq_d, wq_d), ("k", xk_d, wk_d), ("v", xv_d, wv_d)
            ):
                wt = xw.tile([128, KT, DHG], bf16, tag="wT",
                             name=f"wt{which}")
                qeng[which].dma_start(
                    out=wt[:], in_=w_d.rearrange("(n p) s -> p n s", p=128)
                )
                xts[which] = xw.tile([128, KT, S], bf16, tag="xT",
                                     name=f"xt{which}")
                wts[which] = wt
            for s in range(NQS):
                for which, x_d in (("q", xq_d), ("k", xk_d), ("v", xv_d)):
                    xr = x_d.rearrange("(n p) s -> p n s", p=128)
                    qeng[which].dma_start(
                        out=xts[which][:, :, s * QS : (s + 1) * QS],
                        in_=xr[:, :, s * QS : (s + 1) * QS],
                    )
            nc.sync.dma_start(
                out=wo_sb[:], in_=wo_d.rearrange("(n p) d -> p n d", p=128)
            )
            if n_bias:
                nc.sync.dma_start(
                    out=bias_sb[:],
                    in_=bias_d.rearrange("n p a c -> p n a c"),
                )

            # ---- filler task machinery ----
            def qk_group(which, s, mt):
                def emit():
                    dst = qt_sb if which == "q" else kt_sb
                    ps = psS.tile([128, QS], f32, tag="ps",
                                  name=f"pj{which}{s}{mt}")
                    for kt in range(KT):
                        nc.tensor.matmul(
                            ps[:],
                            wts[which][:, kt, mt * 128 : (mt + 1) * 128],
                            xts[which][:, kt, s * QS : (s + 1) * QS],
                            start=(kt == 0),
                            stop=(kt == KT - 1),
                        )
                    nc.vector.tensor_copy(
                        out=dst[:, mt, s * QS : (s + 1) * QS], in_=ps[:]
                    )
                return emit

            def v_group(st):
                def emit():
                    ps = psS.tile([128, QS], f32, tag="ps", name=f"pjv{st}")
                    for kt in range(KT):
                        nc.tensor.matmul(
                            ps[:, :DHG],
                            xts["v"][:, kt, st * 128 : (st + 1) * 128],
                            wts["v"][:, kt, :],
                            start=(kt == 0),
                            stop=(kt == KT - 1),
                        )
                    nc.vector.tensor_copy(
                        out=v_sb[:, st, :, 0:DK],
                        in_=ps[:, :DHG].rearrange("p (h d) -> p h d", h=HG),
                    )
                return emit

            # deadlines: Q_s before (s,0); K_s before the strip's first
            # k-tile inside K-strip s; V_st before PV of kt=st in the first
            # strip whose k-range covers it (PV(idx) emits in slot idx+1)
            kts_of = {qs: strip_kts(cls, qs) for qs in range(NQS)}
            tasks = []
            for s in range(NQS):
                kts = kts_of[s]
                # strip 0: pair-0 weights land before the pair-0 scores of
                # the first k-tile so the exp stream starts ~5us earlier
                mt1_phase = 3 if s == 0 else 1
                tasks.append(((s, 0, 0), qk_group("q", s, 0)))
                tasks.append(((s, 0, mt1_phase), qk_group("q", s, 1)))
                kidx = next((i for i, kt in enumerate(kts)
                             if kt * 128 >= s * QS), 0)
                tasks.append(((s, kidx, 0), qk_group("k", s, 0)))
                tasks.append(((s, kidx, mt1_phase if kidx == 0 else 1),
                              qk_group("k", s, 1)))
            first_use = {}
            for s in range(NQS):
                for i, kt in enumerate(kts_of[s]):
                    if kt not in first_use:
                        first_use[kt] = (s, i + 1, 0)
            for st in range(ST128):
                if st in first_use:
                    tasks.append((first_use[st], v_group(st)))
            tasks.sort(key=lambda t: t[0])

            def pump(now):
                while tasks and tasks[0][0] <= now:
                    tasks.pop(0)[1]()

            def emit_yproj(st):
                yps = psS.tile([128, 2, QS], f32, tag="ps", name=f"yps{st}")
                for nh in range(2):
                    for mt in range(2):
                        nc.tensor.matmul(
                            yps[:, nh, :],
                            ot_sb[:, mt, st * 128 : (st + 1) * 128],
                            wo_sb[:, mt, nh * QS : (nh + 1) * QS],
                            start=(mt == 0),
                            stop=(mt == 1),
                        )
                y_sb = ypool.tile([128, 2 * QS], f32, tag="y",
                                  name=f"ysb{st}")
                nc.vector.tensor_copy(
                    out=y_sb[:], in_=yps[:].rearrange("p a b -> p (a b)")
                )
                nc.gpsimd.dma_start(
                    out=y_d[st * 128 : (st + 1) * 128, :], in_=y_sb[:]
                )

            pend_y = []

            def pop_one():
                if tasks:
                    tasks.pop(0)[1]()
                elif pend_y:
                    emit_yproj(pend_y.pop(0))

            def emit_pv(pots, p_of, c0_of, kts, idx):
                kt = kts[idx]
                c0 = c0_of[kt]
                if idx == 0 and c0 > 0:
                    nc.vector.memset(pots[:, :, 0:c0], 0.0)
                for h in range(HG):
                    nc.tensor.matmul(
                        pots[:, h, c0:],
                        v_sb[:, kt, h, :],
                        p_of[kt][:, h, c0:],
                        start=(idx == 0),
                        stop=(idx == len(kts) - 1),
                    )

            # ---- fused attention + projection strip loop ----
            for qs in range(NQS):
                sub_all = cls[:, 4 * qs : 4 * qs + 4]
                kts = kts_of[qs]
                pots = psO.tile([128, HG, QS], f32, tag="pot",
                                name=f"pot{qs}")
                c0_of = {}
                p_of = {}
                for idx, kt in enumerate(kts):
                    pump((qs, idx, 2))
                    sub = sub_all[kt]
                    nz = np.nonzero(sub)[0]
                    c0 = int(nz.min()) * 128
                    c1 = (int(nz.max()) + 1) * 128
                    c0_of[kt] = c0
                    partial_js = [j for j in range(4) if sub[j] == 2]
                    interior = [
                        j for j in range(4)
                        if sub[j] == 0 and c0 // 128 < j < c1 // 128
                    ]
                    # scores: per pair j one 2-bank psum tile; the pair's
                    # heads (row groups 0:64 / 64:128) run concurrently in
                    # disjoint PE subarrays
                    p_sb = pb.tile([128, HG, QS], bf16, tag="p",
                                   name=f"p{qs}_{kt}")
                    p_of[kt] = p_sb
                    for j in range(2):
                        if j == 1:
                            pump((qs, idx, 4))
                        ps = psS.tile([128, 2, QS], f32, tag="ps",
                                      name=f"pp{qs}_{kt}_{j}")
                        for hh in range(2):
                            po = 64 * hh
                            nc.tensor.matmul(
                                ps[:, hh, c0:c1],
                                kt_sb[po : po + 64, j,
                                      kt * 128 : (kt + 1) * 128],
                                qt_sb[po : po + 64, j,
                                      qs * QS + c0 : qs * QS + c1],
                                start=True,
                                stop=True,
                            )
                        nc.scalar.activation(
                            p_sb[:, 2 * j : 2 * j + 2, c0:c1],
                            ps[:, :, c0:c1],
                            EXP,
                            scale=0.125,
                        )
                    for jj in interior:
                        nc.vector.memset(
                            p_sb[:, :, jj * 128 : (jj + 1) * 128], 0.0
                        )
                    for jj in partial_js:
                        bi = int(bidx[kt, 4 * qs + jj])
                        nc.vector.tensor_mul(
                            p_sb[:, :, jj * 128 : (jj + 1) * 128],
                            p_sb[:, :, jj * 128 : (jj + 1) * 128],
                            bias_sb[:, bi, :, :],
                        )
                    if idx == 1:
                        # strip boundary: the first PV waits on the previous
                        # strip's normalize reads of pots (WAR); give the PE
                        # filler work ahead of it
                        pop_one()
                        pop_one()
                        emit_pv(pots, p_of, c0_of, kts, idx - 1)
                    elif idx >= 2:
                        emit_pv(pots, p_of, c0_of, kts, idx - 1)
                        pop_one()
                pump((qs, len(kts), 2))
                emit_pv(pots, p_of, c0_of, kts, len(kts) - 1)
                # ---- normalize: invD = Exp(-Ln(D)) on ACT (same table set
                # as the exp stream, so no table reloads)
                invd = dn.tile([128, HG, QS], f32, tag="dn", name=f"dn{qs}")
                nc.scalar.activation(
                    invd[64:128, :, :], pots[64:128, :, :], LN
                )
                nc.scalar.activation(
                    invd[64:128, :, :], invd[64:128, :, :], EXP, scale=-1.0
                )
                for h in range(HG):
                    po = 64 * (h % 2)
                    nc.vector.tensor_mul(
                        ot_sb[po : po + 64, h // 2, qs * QS : (qs + 1) * QS],
                        pots[0:DK, h, :],
                        invd[64:128, h, :],
                    )
                for sti in range(QS // 128):
                    pend_y.append(qs * (QS // 128) + sti)
            while tasks or pend_y:
                pop_one()

    _split_waits(nc)
    return nc


_program_cache = {}


def get_program(cls, bidx, n_bias):
    key = (cls.tobytes(), bidx.tobytes(), n_bias)
    if key not in _program_cache:
        _program_cache[key] = build_program(cls, bidx, n_bias)
    return _program_cache[key]


def make_in_maps(q, k, v, mask, w_q, w_k, w_v, w_o, biases):
    if biases:
        # replicate each partial mask block across the four head slots of
        # the p tile so one DVE tensor_mul covers all heads
        bias_arr = np.stack(
            [np.repeat(b[:, None, :], HG, axis=1) for b in biases]
        ).astype(ml_dtypes.bfloat16)
    else:
        bias_arr = np.zeros((1, 128, HG, 128), ml_dtypes.bfloat16)
    in_maps = []
    for c in range(NCORES):
        b, g = divmod(c, 4)
        rows = slice(g * DHG, (g + 1) * DHG)
        in_maps.append(
            {
                "xqT": np.ascontiguousarray(q[b].T).astype(ml_dtypes.bfloat16),
                "xkT": np.ascontiguousarray(k[b].T).astype(ml_dtypes.bfloat16),
                "xvT": np.ascontiguousarray(v[b].T).astype(ml_dtypes.bfloat16),
                "wqT": np.ascontiguousarray(w_q[rows].T).astype(
                    ml_dtypes.bfloat16
                ),
                "wkT": np.ascontiguousarray(w_k[rows].T).astype(
                    ml_dtypes.bfloat16
                ),
                "wvT": np.ascontiguousarray(w_v[rows].T).astype(
                    ml_dtypes.bfloat16
                ),
                "woT": np.ascontiguousarray(w_o[:, rows].T).astype(
                    ml_dtypes.bfloat16
                ),
                "biasT": bias_arr,
            }
        )
    return in_maps


def combine_results(results):
    out = np.empty((B, S, D), np.float32)
    for b in range(B):
        acc = results[4 * b]["y"].astype(np.float32).copy()
        for g in range(1, 4):
            acc += results[4 * b + g]["y"]
        out[b] = acc
    return out


def kernel(q, k, v, mask, w_q, w_k, w_v, w_o):
    q = np.asarray(q, np.float32)
    k = np.asarray(k, np.float32)
    v = np.asarray(v, np.float32)
    w_q = np.asarray(w_q, np.float32)
    w_k = np.asarray(w_k, np.float32)
    w_v = np.asarray(w_v, np.float32)
    w_o = np.asarray(w_o, np.float32)
    maskT = np.ascontiguousarray(
        np.broadcast_to(np.asarray(mask), (1, 1, S, S))[0, 0].T
    )
    cls, bidx, biases = classify_mask(maskT)
    nc = get_program(cls, bidx, len(biases))
    in_maps = make_in_maps(q, k, v, mask, w_q, w_k, w_v, w_o, biases)
    res = run_bass_kernel_spmd(nc, in_maps, list(range(NCORES)))
    return combine_results(res.results)


# revision 18
# speedup vs baseline: 1.0074x; 1.0074x over previous
"""Multi-head attention (B=2, S=2048, D=1024, H=16) on 8 trn2 NeuronCores.

Sharding: batch (2) x head-groups (4 heads each, 4 groups) = 8 cores.
Each core computes Q/K/V projections for its 4 heads on its batch,
causal-masked softmax attention, and a partial output projection
(row-sharded w_o); the host sums the 4 partials per batch.

Layout strategy: the host stages transposed inputs (xT = x[b].T) so every
matmul contraction runs over the SBUF partition axis with no on-device
transposes. Attention scores are computed transposed (ST[k, q]) so that
P = exp(ST) serves directly as the PV matmul's moving operand.

v3 structure (269us -> 207us -> this):
- Scores run un-padded K=64 with two heads packed concurrently in the PE
  array via base_partition 0/64 (tile_position row groups).
- The PV stationary operand is [V_h | 64 columns of ones]: output rows
  0:64 are the attention output, rows 64:128 all accumulate the softmax
  denominator D (broadcast by the matmul for free). The normalizer
  1/D = Exp(-Ln(D)) runs on the Activation engine (both functions live in
  the natural_log_exp_and_others table - one load, no switches).
- There is no separate projection phase: Q/K/V projection matmul groups
  are "filler" tasks with emission deadlines, woven into the attention
  strip loop wherever the (ACT-bound) attention chain leaves PE slack,
  alongside the previous strip's output-projection tiles. Scores for
  k-tile N+1 are issued before PV of k-tile N so the PE FIFO never
  head-of-line blocks on exp. The PE therefore stays continuously busy
  and the HAM clock gate stays at 2.4 GHz.
- x^T input DMAs are issued on the gpsimd (software DGE) queue, strip-
  interleaved q0,k0,v0,q1,... so the first projection group can start
  ~5us in; weight DMAs ride the sync queue; y output DMAs share gpsimd.
- PSUM: tag-shared ring of 2x 4KiB slots (score pair tiles [128,2,512]f32,
  y-projection tiles, and 2KiB projection tiles all rotate through it)
  + pots [128,4,512]f32 (4 banks) = 8 banks exactly.
"""
import sys

sys.path.insert(0, "/opt/trn_rl_repo")

import numpy as np
import ml_dtypes

import concourse.bass as bass
import concourse.mybir as mybir
import concourse.tile as tile
from concourse.bass_utils import run_bass_kernel_spmd

B, S, D, H, DK = 2, 2048, 1024, 16, 64
NCORES = 8
HG = 4                # heads per core
DHG = HG * DK         # 256 head-dims per core
KT = D // 128         # 8 contraction tiles for the projections
ST128 = S // 128      # 16 128-row tiles of S
QS = 512              # q-strip width
NQS = S // QS         # 4 strips

f32 = mybir.dt.float32
bf16 = mybir.dt.bfloat16
EXP = mybir.ActivationFunctionType.Exp
LN = mybir.ActivationFunctionType.Ln


def _split_waits(nc, max_waits=1):
    """This walrus build rejects >1 SyncWait per instruction (and >0 on
    fp32-family matmuls, which lower through the 1-wait S3_LW struct).
    Hoist excess waits onto dedicated NOPs on the same engine queue."""
    n = 0
    for fn in nc.m.functions:
        for blk in fn.blocks:
            new = []
            for ins in blk.instructions:
                si = getattr(ins, "sync_info", None)
                if si is not None and si.on_wait:
                    limit = 0 if isinstance(ins, mybir.InstMatmult) else max_waits
                    if len(si.on_wait) > limit:
                        waits = list(si.on_wait)
                        hoist = waits if limit == 0 else waits[:-limit]
                        keep = [] if limit == 0 else waits[-limit:]
                        for w in hoist:
                            n += 1
                            new.append(
                                mybir.InstNoOp(
                                    name=f"I-waitfix-{n}",
                                    engine=ins.engine,
                                    bass_nofuse=True,
                                    sync_info=mybir.SyncInfo(
                                        on_wait=[w], on_update=[]
                                    ),
                                )
                            )
                        ins.sync_info = mybir.SyncInfo(
                            on_wait=keep, on_update=list(si.on_update)
                        )
                new.append(ins)
            blk.instructions[:] = new
    return n


def classify_mask(maskT):
    """Block-classify the transposed mask at 128x128 granularity.
    Returns (cls[i,j] in {0 empty,1 full,2 partial}, bias index map,
    list of multiplicative fp32 mask blocks for the partial ones)."""
    nb = S // 128
    cls = np.empty((nb, nb), dtype=np.int8)
    bidx = np.full((nb, nb), -1, dtype=np.int32)
    biases = []
    for i in range(nb):
        for j in range(nb):
            blk = maskT[i * 128 : (i + 1) * 128, j * 128 : (j + 1) * 128]
            if (blk != 0).all():
                cls[i, j] = 1
            elif (blk == 0).all():
                cls[i, j] = 0
            else:
                cls[i, j] = 2
                bidx[i, j] = len(biases)
                biases.append((blk != 0).astype(np.float32))
    return cls, bidx, biases


def strip_kts(cls, qs):
    sub_all = cls[:, 4 * qs : 4 * qs + 4]
    return [i for i in range(ST128) if sub_all[i].any()]


def build_program(cls, bidx, n_bias):
    nb_alloc = max(1, n_bias)
    nc = bass.Bass("TRN2", target_bir_lowering=False, debug=False,
                   num_devices=NCORES)
    xq_d = nc.dram_tensor("xqT", [D, S], bf16, kind="ExternalInput").ap()
    xk_d = nc.dram_tensor("xkT", [D, S], bf16, kind="ExternalInput").ap()
    xv_d = nc.dram_tensor("xvT", [D, S], bf16, kind="ExternalInput").ap()
    wq_d = nc.dram_tensor("wqT", [D, DHG], bf16, kind="ExternalInput").ap()
    wk_d = nc.dram_tensor("wkT", [D, DHG], bf16, kind="ExternalInput").ap()
    wv_d = nc.dram_tensor("wvT", [D, DHG], bf16, kind="ExternalInput").ap()
    wo_d = nc.dram_tensor("woT", [DHG, D], bf16, kind="ExternalInput").ap()
    bias_d = nc.dram_tensor("biasT", [nb_alloc, 128, HG, 128], bf16,
                            kind="ExternalInput").ap()
    y_d = nc.dram_tensor("y", [S, D], f32, kind="ExternalOutput").ap()

    with tile.TileContext(nc) as tc:
        with tc.tile_pool(name="persist", bufs=1) as pp, tc.tile_pool(
            name="xw", bufs=3
        ) as xw, tc.tile_pool(
            name="pb", bufs=4
        ) as pb, tc.tile_pool(
            name="dn", bufs=1
        ) as dn, tc.tile_pool(
            name="yb", bufs=2
        ) as ypool, tc.tile_pool(
            name="psS", bufs=2, space="PSUM"
        ) as psS, tc.tile_pool(
            name="psO", bufs=1, space="PSUM"
        ) as psO:
            # pair-major head layout: partitions 0:64 <-> head 2j, 64:128
            # <-> head 2j+1, for pair index j in {0,1}
            qt_sb = pp.tile([128, 2, S], bf16)            # Q^T
            kt_sb = pp.tile([128, 2, S], bf16)            # K^T
            v_sb = pp.tile([128, ST128, HG, 128], bf16)   # [V | ones]
            ot_sb = pp.tile([128, 2, S], bf16)            # attn out^T
            wo_sb = pp.tile([128, 2, D], bf16)
            bias_sb = pp.tile([128, nb_alloc, HG, 128], bf16)
            # denominator-broadcast columns of the PV stationary operand
            # (DVE is idle this early; gpsimd's queue is needed for DMAs)
            nc.vector.memset(v_sb[:, :, :, DK:], 1.0)

            # ---- input DMAs ----
            # x^T rows are 4KiB contiguous in DRAM; strip-slicing fragments
            # them into 1KiB packets whose per-packet overhead caps each
            # queue at ~120GB/s. So only strip 0 (needed in the first ~10us)
            # is sliced; strips 1:4 of each tensor move as one DMA with 3KiB
            # runs. Strip-0 slices + xv ride the gpsimd software-DGE queue
            # (cheap triggers); weights + the big xq/xk remainders ride
            # sync. The scalar queue stays clean for the exp stream.
            xts = {}
            wts = {}
            for which, x_d, w_d in (
                ("q", xq_d, wq_d), ("k", xk_d, wk_d), ("v", xv_d, wv_d)
            ):
                wt = xw.tile([128, KT, DHG], bf16, tag="wT",
                             name=f"wt{which}")
                nc.sync.dma_start(
                    out=wt[:], in_=w_d.rearrange("(n p) s -> p n s", p=128)
                )
                xts[which] = xw.tile([128, KT, S], bf16, tag="xT",
                                     name=f"xt{which}")
                wts[which] = wt
            for which, x_d in (("q", xq_d), ("k", xk_d), ("v", xv_d)):
                xr = x_d.rearrange("(n p) s -> p n s", p=128)
                nc.gpsimd.dma_start(
                    out=xts[which][:, :, 0:QS], in_=xr[:, :, 0:QS]
                )
            nc.sync.dma_start(
                out=wo_sb[:], in_=wo_d.rearrange("(n p) d -> p n d", p=128)
            )
            for which, x_d, eng in (
                ("q", xq_d, nc.sync), ("k", xk_d, nc.sync),
                ("v", xv_d, nc.gpsimd),
            ):
                xr = x_d.rearrange("(n p) s -> p n s", p=128)
                eng.dma_start(
                    out=xts[which][:, :, QS:], in_=xr[:, :, QS:]
                )
            if n_bias:
                nc.sync.dma_start(
                    out=bias_sb[:],
                    in_=bias_d.rearrange("n p a c -> p n a c"),
                )

            # ---- filler task machinery ----
            def qk_group(which, s, mt):
                def emit():
                    dst = qt_sb if which == "q" else kt_sb
                    ps = psS.tile([128, QS], f32, tag="ps",
                                  name=f"pj{which}{s}{mt}")
                    for kt in range(KT):
                        nc.tensor.matmul(
                            ps[:],
                            wts[which][:, kt, mt * 128 : (mt + 1) * 128],
                            xts[which][:, kt, s * QS : (s + 1) * QS],
                            start=(kt == 0),
                            stop=(kt == KT - 1),
                        )
                    nc.vector.tensor_copy(
                        out=dst[:, mt, s * QS : (s + 1) * QS], in_=ps[:]
                    )
                return emit

            def v_group(st):
                def emit():
                    ps = psS.tile([128, QS], f32, tag="ps", name=f"pjv{st}")
                    for kt in range(KT):
                        nc.tensor.matmul(
                            ps[:, :DHG],
                            xts["v"][:, kt, st * 128 : (st + 1) * 128],
                            wts["v"][:, kt, :],
                            start=(kt == 0),
                            stop=(kt == KT - 1),
                        )
                    nc.vector.tensor_copy(
                        out=v_sb[:, st, :, 0:DK],
                        in_=ps[:, :DHG].rearrange("p (h d) -> p h d", h=HG),
                    )
                return emit

            # deadlines: Q_s before (s,0); K_s before the strip's first
            # k-tile inside K-strip s; V_st before PV of kt=st in the first
            # strip whose k-range covers it (PV(idx) emits in slot idx+1)
            kts_of = {qs: strip_kts(cls, qs) for qs in range(NQS)}
            tasks = []
            for s in range(NQS):
                kts = kts_of[s]
                # strip 0: pair-0 weights land before the pair-0 scores of
                # the first k-tile so the exp stream starts ~5us earlier
                mt1_phase = 3 if s == 0 else 1
                tasks.append(((s, 0, 0), qk_group("q", s, 0)))
                tasks.append(((s, 0, mt1_phase), qk_group("q", s, 1)))
                kidx = next((i for i, kt in enumerate(kts)
                             if kt * 128 >= s * QS), 0)
                tasks.append(((s, kidx, 0), qk_group("k", s, 0)))
                tasks.append(((s, kidx, mt1_phase if kidx == 0 else 1),
                              qk_group("k", s, 1)))
            first_use = {}
            for s in range(NQS):
                for i, kt in enumerate(kts_of[s]):
                    if kt not in first_use:
                        first_use[kt] = (s, i + 1, 0)
            for st in range(ST128):
                if st in first_use:
                    tasks.append((first_use[st], v_group(st)))
            tasks.sort(key=lambda t: t[0])

            def pump(now):
                while tasks and tasks[0][0] <= now:
                    tasks.pop(0)[1]()

            def emit_yproj(st):
                yps = psS.tile([128, 2, QS], f32, tag="ps", name=f"yps{st}")
                for nh in range(2):
                    for mt in range(2):
                        nc.tensor.matmul(
                            yps[:, nh, :],
                            ot_sb[:, mt, st * 128 : (st + 1) * 128],
                            wo_sb[:, mt, nh * QS : (nh + 1) * QS],
                            start=(mt == 0),
                            stop=(mt == 1),
                        )
                y_sb = ypool.tile([128, 2 * QS], f32, tag="y",
                                  name=f"ysb{st}")
                nc.vector.tensor_copy(
                    out=y_sb[:], in_=yps[:].rearrange("p a b -> p (a b)")
                )
                nc.gpsimd.dma_start(
                    out=y_d[st * 128 : (st + 1) * 128, :], in_=y_sb[:]
                )

            pend_y = []

            def pop_one():
                if tasks:
                    tasks.pop(0)[1]()
                elif pend_y:
                    emit_yproj(pend_y.pop(0))

            def emit_pv(pots, p_of, c0_of, kts, idx):
                kt = kts[idx]
                c0 = c0_of[kt]
                if idx == 0 and c0 > 0:
                    nc.vector.memset(pots[:, :, 0:c0], 0.0)
                for h in range(HG):
                    nc.tensor.matmul(
                        pots[:, h, c0:],
                        v_sb[:, kt, h, :],
                        p_of[kt][:, h, c0:],
                        start=(idx == 0),
                        stop=(idx == len(kts) - 1),
                    )

            # ---- fused attention + projection strip loop ----
            for qs in range(NQS):
                sub_all = cls[:, 4 * qs : 4 * qs + 4]
                kts = kts_of[qs]
                pots = psO.tile([128, HG, QS], f32, tag="pot",
                                name=f"pot{qs}")
                c0_of = {}
                p_of = {}
                for idx, kt in enumerate(kts):
                    pump((qs, idx, 2))
                    sub = sub_all[kt]
                    nz = np.nonzero(sub)[0]
                    c0 = int(nz.min()) * 128
                    c1 = (int(nz.max()) + 1) * 128
                    c0_of[kt] = c0
                    partial_js = [j for j in range(4) if sub[j] == 2]
                    interior = [
                        j for j in range(4)
                        if sub[j] == 0 and c0 // 128 < j < c1 // 128
                    ]
                    # scores: per pair j one 2-bank psum tile; the pair's
                    # heads (row groups 0:64 / 64:128) run concurrently in
                    # disjoint PE subarrays
                    p_sb = pb.tile([128, HG, QS], bf16, tag="p",
                                   name=f"p{qs}_{kt}")
                    p_of[kt] = p_sb
                    for j in range(2):
                        if j == 1:
                            pump((qs, idx, 4))
                        ps = psS.tile([128, 2, QS], f32, tag="ps",
                                      name=f"pp{qs}_{kt}_{j}")
                        for hh in range(2):
                            po = 64 * hh
                            nc.tensor.matmul(
                                ps[:, hh, c0:c1],
                                kt_sb[po : po + 64, j,
                                      kt * 128 : (kt + 1) * 128],
                                qt_sb[po : po + 64, j,
                                      qs * QS + c0 : qs * QS + c1],
                                start=True,
                                stop=True,
                            )
                        nc.scalar.activation(
                            p_sb[:, 2 * j : 2 * j + 2, c0:c1],
                            ps[:, :, c0:c1],
                            EXP,
                            scale=0.125,
                        )
                    for jj in interior:
                        nc.vector.memset(
                            p_sb[:, :, jj * 128 : (jj + 1) * 128], 0.0
                        )
                    for jj in partial_js:
                        bi = int(bidx[kt, 4 * qs + jj])
                        nc.vector.tensor_mul(
                            p_sb[:, :, jj * 128 : (jj + 1) * 128],
                            p_sb[:, :, jj * 128 : (jj + 1) * 128],
                            bias_sb[:, bi, :, :],
                        )
                    if idx == 1:
                        # strip boundary: the first PV waits on the previous
                        # strip's normalize reads of pots (WAR); give the PE
                        # filler work ahead of it
                        pop_one()
                        pop_one()
                        emit_pv(pots, p_of, c0_of, kts, idx - 1)
                    elif idx >= 2:
                        emit_pv(pots, p_of, c0_of, kts, idx - 1)
                        pop_one()
                        if qs >= 2:
                            # late strips are ACT-bound: drain the y-proj
                            # backlog into the PE slack instead of a tail
                            pop_one()
                pump((qs, len(kts), 2))
                emit_pv(pots, p_of, c0_of, kts, len(kts) - 1)
                # ---- normalize: invD = Exp(-Ln(D)) on ACT (same table set
                # as the exp stream, so no table reloads)
                invd = dn.tile([128, HG, QS], f32, tag="dn", name=f"dn{qs}")
                nc.scalar.activation(
                    invd[64:128, :, :], pots[64:128, :, :], LN
                )
                nc.scalar.activation(
                    invd[64:128, :, :], invd[64:128, :, :], EXP, scale=-1.0
                )
                for h in range(HG):
                    po = 64 * (h % 2)
                    nc.vector.tensor_mul(
                        ot_sb[po : po + 64, h // 2, qs * QS : (qs + 1) * QS],
                        pots[0:DK, h, :],
                        invd[64:128, h, :],
                    )
                for sti in range(QS // 128):
                    pend_y.append(qs * (QS // 128) + sti)
            while tasks or pend_y:
                pop_one()

    _split_waits(nc)
    return nc


_program_cache = {}


def get_program(cls, bidx, n_bias):
    key = (cls.tobytes(), bidx.tobytes(), n_bias)
    if key not in _program_cache:
        _program_cache[key] = build_program(cls, bidx, n_bias)
    return _program_cache[key]


def make_in_maps(q, k, v, mask, w_q, w_k, w_v, w_o, biases):
    if biases:
        # replicate each partial mask block across the four head slots of
        # the p tile so one DVE tensor_mul covers all heads
        bias_arr = np.stack(
            [np.repeat(b[:, None, :], HG, axis=1) for b in biases]
        ).astype(ml_dtypes.bfloat16)
    else:
        bias_arr = np.zeros((1, 128, HG, 128), ml_dtypes.bfloat16)
    in_maps = []
    for c in range(NCORES):
        b, g = divmod(c, 4)
        rows = slice(g * DHG, (g + 1) * DHG)
        in_maps.append(
            {
                "xqT": np.ascontiguousarray(q[b].T).astype(ml_dtypes.bfloat16),
                "xkT": np.ascontiguousarray(k[b].T).astype(ml_dtypes.bfloat16),
                "xvT": np.ascontiguousarray(v[b].T).astype(ml_dtypes.bfloat16),
                "wqT": np.ascontiguousarray(w_q[rows].T).astype(
                    ml_dtypes.bfloat16
                ),
                "wkT": np.ascontiguousarray(w_k[rows].T).astype(
                    ml_dtypes.bfloat16
                ),
                "wvT": np.ascontiguousarray(w_v[rows].T).astype(
                    ml_dtypes.bfloat16
                ),
                "woT": np.ascontiguousarray(w_o[:, rows].T).astype(
                    ml_dtypes.bfloat16
                ),
                "biasT": bias_arr,
            }
        )
    return in_maps


def combine_results(results):
    out = np.empty((B, S, D), np.float32)
    for b in range(B):
        acc = results[4 * b]["y"].astype(np.float32).copy()
        for g in range(1, 4):
            acc += results[4 * b + g]["y"]
        out[b] = acc
    return out


def kernel(q, k, v, mask, w_q, w_k, w_v, w_o):
    q = np.asarray(q, np.float32)
    k = np.asarray(k, np.float32)
    v = np.asarray(v, np.float32)
    w_q = np.asarray(w_q, np.float32)
    w_k = np.asarray(w_k, np.float32)
    w_v = np.asarray(w_v, np.float32)
    w_o = np.asarray(w_o, np.float32)
    maskT = np.ascontiguousarray(
        np.broadcast_to(np.asarray(mask), (1, 1, S, S))[0, 0].T
    )
    cls, bidx, biases = classify_mask(maskT)
    nc = get_program(cls, bidx, len(biases))
    in_maps = make_in_maps(q, k, v, mask, w_q, w_k, w_v, w_o, biases)
    res = run_bass_kernel_spmd(nc, in_maps, list(range(NCORES)))
    return combine_results(res.results)


# revision 24
# speedup vs baseline: 1.0496x; 1.0419x over previous
"""Multi-head attention (B=2, S=2048, D=1024, H=16) on 8 trn2 NeuronCores.

Sharding: batch (2) x head-groups (4 heads each, 4 groups) = 8 cores.
Each core computes Q/K/V projections for its 4 heads on its batch,
causal-masked softmax attention, and a partial output projection
(row-sharded w_o); the host sums the 4 partials per batch.

Layout strategy: the host stages transposed inputs (xT = x[b].T) so every
matmul contraction runs over the SBUF partition axis with no on-device
transposes. Attention scores are computed transposed (ST[k, q]) so that
P = exp(ST) serves directly as the PV matmul's moving operand.

v3 structure (269us -> 207us -> this):
- Scores run un-padded K=64 with two heads packed concurrently in the PE
  array via base_partition 0/64 (tile_position row groups).
- The PV stationary operand is [V_h | 64 columns of ones]: output rows
  0:64 are the attention output, rows 64:128 all accumulate the softmax
  denominator D (broadcast by the matmul for free). The normalizer
  1/D = Exp(-Ln(D)) runs on the Activation engine (both functions live in
  the natural_log_exp_and_others table - one load, no switches).
- There is no separate projection phase: Q/K/V projection matmul groups
  are "filler" tasks with emission deadlines, woven into the attention
  strip loop wherever the (ACT-bound) attention chain leaves PE slack,
  alongside the previous strip's output-projection tiles. Scores for
  k-tile N+1 are issued before PV of k-tile N so the PE FIFO never
  head-of-line blocks on exp. The PE therefore stays continuously busy
  and the HAM clock gate stays at 2.4 GHz.
- x^T input DMAs are issued on the gpsimd (software DGE) queue, strip-
  interleaved q0,k0,v0,q1,... so the first projection group can start
  ~5us in; weight DMAs ride the sync queue; y output DMAs share gpsimd.
- PSUM: tag-shared ring of 2x 4KiB slots (score pair tiles [128,2,512]f32,
  y-projection tiles, and 2KiB projection tiles all rotate through it)
  + pots [128,4,512]f32 (4 banks) = 8 banks exactly.
"""
import sys

sys.path.insert(0, "/opt/trn_rl_repo")

import numpy as np
import ml_dtypes

import concourse.bass as bass
import concourse.mybir as mybir
import concourse.tile as tile
from concourse.bass_utils import run_bass_kernel_spmd

B, S, D, H, DK = 2, 2048, 1024, 16, 64
NCORES = 8
HG = 4                # heads per core
DHG = HG * DK         # 256 head-dims per core
KT = D // 128         # 8 contraction tiles for the projections
ST128 = S // 128      # 16 128-row tiles of S
QS = 512              # q-strip width
NQS = S // QS         # 4 strips

f32 = mybir.dt.float32
bf16 = mybir.dt.bfloat16
EXP = mybir.ActivationFunctionType.Exp
LN = mybir.ActivationFunctionType.Ln


def _split_waits(nc, max_waits=1):
    """This walrus build rejects >1 SyncWait per instruction (and >0 on
    fp32-family matmuls, which lower through the 1-wait S3_LW struct).
    Hoist excess waits onto dedicated NOPs on the same engine queue."""
    n = 0
    for fn in nc.m.functions:
        for blk in fn.blocks:
            new = []
            for ins in blk.instructions:
                si = getattr(ins, "sync_info", None)
                if si is not None and si.on_wait:
                    limit = 0 if isinstance(ins, mybir.InstMatmult) else max_waits
                    if len(si.on_wait) > limit:
                        waits = list(si.on_wait)
                        hoist = waits if limit == 0 else waits[:-limit]
                        keep = [] if limit == 0 else waits[-limit:]
                        for w in hoist:
                            n += 1
                            new.append(
                                mybir.InstNoOp(
                                    name=f"I-waitfix-{n}",
                                    engine=ins.engine,
                                    bass_nofuse=True,
                                    sync_info=mybir.SyncInfo(
                                        on_wait=[w], on_update=[]
                                    ),
                                )
                            )
                        ins.sync_info = mybir.SyncInfo(
                            on_wait=keep, on_update=list(si.on_update)
                        )
                new.append(ins)
            blk.instructions[:] = new
    return n


def classify_mask(maskT):
    """Block-classify the transposed mask at 128x128 granularity.
    Returns (cls[i,j] in {0 empty,1 full,2 partial}, bias index map,
    list of multiplicative fp32 mask blocks for the partial ones)."""
    nb = S // 128
    cls = np.empty((nb, nb), dtype=np.int8)
    bidx = np.full((nb, nb), -1, dtype=np.int32)
    biases = []
    for i in range(nb):
        for j in range(nb):
            blk = maskT[i * 128 : (i + 1) * 128, j * 128 : (j + 1) * 128]
            if (blk != 0).all():
                cls[i, j] = 1
            elif (blk == 0).all():
                cls[i, j] = 0
            else:
                cls[i, j] = 2
                bidx[i, j] = len(biases)
                biases.append((blk != 0).astype(np.float32))
    return cls, bidx, biases


def strip_kts(cls, qs):
    sub_all = cls[:, 4 * qs : 4 * qs + 4]
    return [i for i in range(ST128) if sub_all[i].any()]


def build_program(cls, bidx, n_bias):
    nb_alloc = max(1, n_bias)
    nc = bass.Bass("TRN2", target_bir_lowering=False, debug=False,
                   num_devices=NCORES)
    # x^T and w^T are host-staged into the SBUF tile layouts so every DMA
    # row is >=3KiB contiguous (1KiB packets cap a queue at ~120GB/s)
    xq_d = nc.dram_tensor("xqT", [128, KT, S], bf16, kind="ExternalInput").ap()
    xk_d = nc.dram_tensor("xkT", [128, KT, S], bf16, kind="ExternalInput").ap()
    xv_d = nc.dram_tensor("xvT", [128, KT, S], bf16, kind="ExternalInput").ap()
    wq_d = nc.dram_tensor("wqT", [128, KT, DHG], bf16,
                          kind="ExternalInput").ap()
    wk_d = nc.dram_tensor("wkT", [128, KT, DHG], bf16,
                          kind="ExternalInput").ap()
    wv_d = nc.dram_tensor("wvT", [128, KT, DHG], bf16,
                          kind="ExternalInput").ap()
    wo_d = nc.dram_tensor("woT", [128, 2, D], bf16, kind="ExternalInput").ap()
    bias_d = nc.dram_tensor("biasT", [nb_alloc, 128, HG, 128], bf16,
                            kind="ExternalInput").ap()
    y_d = nc.dram_tensor("y", [S, D], bf16, kind="ExternalOutput").ap()

    with tile.TileContext(nc) as tc:
        with tc.tile_pool(name="persist", bufs=1) as pp, tc.tile_pool(
            name="xw", bufs=3
        ) as xw, tc.tile_pool(
            name="pb", bufs=4
        ) as pb, tc.tile_pool(
            name="dn", bufs=1
        ) as dn, tc.tile_pool(
            name="yb", bufs=2
        ) as ypool, tc.tile_pool(
            name="psS", bufs=2, space="PSUM"
        ) as psS, tc.tile_pool(
            name="psO", bufs=1, space="PSUM"
        ) as psO:
            # pair-major head layout: partitions 0:64 <-> head 2j, 64:128
            # <-> head 2j+1, for pair index j in {0,1}
            qt_sb = pp.tile([128, 2, S], bf16)            # Q^T
            kt_sb = pp.tile([128, 2, S], bf16)            # K^T
            v_sb = pp.tile([128, ST128, HG, 128], bf16)   # [V | ones]
            ot_sb = pp.tile([128, 2, S], bf16)            # attn out^T
            wo_sb = pp.tile([128, 2, D], bf16)
            bias_sb = pp.tile([128, nb_alloc, HG, 128], bf16)
            # denominator-broadcast columns of the PV stationary operand
            # (DVE is idle this early; gpsimd's queue is needed for DMAs)
            nc.vector.memset(v_sb[:, :, :, DK:], 1.0)

            # ---- input DMAs ----
            # Arrival order is engineered to match the fused schedule's
            # demand order: strip-0 x slices land first (gpsimd queue),
            # weights almost immediately (sync, now 4KiB rows), then
            # xq_rest (sync) for strip 1's Q projection, xk_rest and
            # xv_rest (gpsimd) for the later k-tiles. The scalar queue
            # stays clean for the exp stream.
            xts = {}
            wts = {}
            for which, x_d, w_d in (
                ("q", xq_d, wq_d), ("k", xk_d, wk_d), ("v", xv_d, wv_d)
            ):
                wt = xw.tile([128, KT, DHG], bf16, tag="wT",
                             name=f"wt{which}")
                nc.sync.dma_start(out=wt[:], in_=w_d)
                xts[which] = xw.tile([128, KT, S], bf16, tag="xT",
                                     name=f"xt{which}")
                wts[which] = wt
            for which, x_d in (("q", xq_d), ("k", xk_d), ("v", xv_d)):
                nc.gpsimd.dma_start(
                    out=xts[which][:, :, 0:QS], in_=x_d[:, :, 0:QS]
                )
            nc.sync.dma_start(out=wo_sb[:], in_=wo_d)
            if n_bias:
                nc.sync.dma_start(
                    out=bias_sb[:],
                    in_=bias_d.rearrange("n p a c -> p n a c"),
                )
            nc.sync.dma_start(
                out=xts["q"][:, :, QS:], in_=xq_d[:, :, QS:]
            )
            nc.gpsimd.dma_start(
                out=xts["k"][:, :, QS:], in_=xk_d[:, :, QS:]
            )
            nc.gpsimd.dma_start(
                out=xts["v"][:, :, QS:], in_=xv_d[:, :, QS:]
            )

            # ---- filler task machinery ----
            def qk_group(which, s, mt):
                def emit():
                    dst = qt_sb if which == "q" else kt_sb
                    ps = psS.tile([128, QS], f32, tag="ps",
                                  name=f"pj{which}{s}{mt}")
                    for kt in range(KT):
                        nc.tensor.matmul(
                            ps[:],
                            wts[which][:, kt, mt * 128 : (mt + 1) * 128],
                            xts[which][:, kt, s * QS : (s + 1) * QS],
                            start=(kt == 0),
                            stop=(kt == KT - 1),
                        )
                    nc.vector.tensor_copy(
                        out=dst[:, mt, s * QS : (s + 1) * QS], in_=ps[:]
                    )
                return emit

            def v_group(st):
                def emit():
                    ps = psS.tile([128, QS], f32, tag="ps", name=f"pjv{st}")
                    for kt in range(KT):
                        nc.tensor.matmul(
                            ps[:, :DHG],
                            xts["v"][:, kt, st * 128 : (st + 1) * 128],
                            wts["v"][:, kt, :],
                            start=(kt == 0),
                            stop=(kt == KT - 1),
                        )
                    nc.vector.tensor_copy(
                        out=v_sb[:, st, :, 0:DK],
                        in_=ps[:, :DHG].rearrange("p (h d) -> p h d", h=HG),
                    )
                return emit

            # deadlines: Q_s before (s,0); K_s before the strip's first
            # k-tile inside K-strip s; V_st before PV of kt=st in the first
            # strip whose k-range covers it (PV(idx) emits in slot idx+1)
            kts_of = {qs: strip_kts(cls, qs) for qs in range(NQS)}
            tasks = []
            for s in range(NQS):
                kts = kts_of[s]
                # strip 0: pair-0 weights land before the pair-0 scores of
                # the first k-tile so the exp stream starts ~5us earlier
                mt1_phase = 3 if s == 0 else 1
                tasks.append(((s, 0, 0), qk_group("q", s, 0)))
                tasks.append(((s, 0, mt1_phase), qk_group("q", s, 1)))
                kidx = next((i for i, kt in enumerate(kts)
                             if kt * 128 >= s * QS), 0)
                tasks.append(((s, kidx, 0), qk_group("k", s, 0)))
                tasks.append(((s, kidx, mt1_phase if kidx == 0 else 1),
                              qk_group("k", s, 1)))
            first_use = {}
            for s in range(NQS):
                for i, kt in enumerate(kts_of[s]):
                    if kt not in first_use:
                        first_use[kt] = (s, i + 1, 0)
            for st in range(ST128):
                if st in first_use:
                    tasks.append((first_use[st], v_group(st)))
            tasks.sort(key=lambda t: t[0])

            def pump(now):
                while tasks and tasks[0][0] <= now:
                    tasks.pop(0)[1]()

            def emit_yproj(st):
                yps = psS.tile([128, 2, QS], f32, tag="ps", name=f"yps{st}")
                for nh in range(2):
                    for mt in range(2):
                        nc.tensor.matmul(
                            yps[:, nh, :],
                            ot_sb[:, mt, st * 128 : (st + 1) * 128],
                            wo_sb[:, mt, nh * QS : (nh + 1) * QS],
                            start=(mt == 0),
                            stop=(mt == 1),
                        )
                y_sb = ypool.tile([128, 2 * QS], bf16, tag="y",
                                  name=f"ysb{st}")
                nc.vector.tensor_copy(
                    out=y_sb[:], in_=yps[:].rearrange("p a b -> p (a b)")
                )
                nc.gpsimd.dma_start(
                    out=y_d[st * 128 : (st + 1) * 128, :], in_=y_sb[:]
                )

            pend_y = []

            def pop_one():
                # y-projections first: they depend only on completed strips
                # (never on a pending input DMA), so they can never stall
                # the PE FIFO; projection groups are pulled by their
                # deadline pumps
                if pend_y:
                    emit_yproj(pend_y.pop(0))
                elif tasks:
                    tasks.pop(0)[1]()

            def emit_pv(pots, p_of, c0_of, kts, idx):
                kt = kts[idx]
                c0 = c0_of[kt]
                if idx == 0 and c0 > 0:
                    nc.vector.memset(pots[:, :, 0:c0], 0.0)
                for h in range(HG):
                    nc.tensor.matmul(
                        pots[:, h, c0:],
                        v_sb[:, kt, h, :],
                        p_of[kt][:, h, c0:],
                        start=(idx == 0),
                        stop=(idx == len(kts) - 1),
                    )

            # ---- fused attention + projection strip loop ----
            for qs in range(NQS):
                sub_all = cls[:, 4 * qs : 4 * qs + 4]
                kts = kts_of[qs]
                pots = psO.tile([128, HG, QS], f32, tag="pot",
                                name=f"pot{qs}")
                c0_of = {}
                p_of = {}
                for idx, kt in enumerate(kts):
                    pump((qs, idx, 2))
                    sub = sub_all[kt]
                    nz = np.nonzero(sub)[0]
                    c0 = int(nz.min()) * 128
                    c1 = (int(nz.max()) + 1) * 128
                    c0_of[kt] = c0
                    partial_js = [j for j in range(4) if sub[j] == 2]
                    interior = [
                        j for j in range(4)
                        if sub[j] == 0 and c0 // 128 < j < c1 // 128
                    ]
                    # scores: per pair j one 2-bank psum tile; the pair's
                    # heads (row groups 0:64 / 64:128) run concurrently in
                    # disjoint PE subarrays
                    p_sb = pb.tile([128, HG, QS], bf16, tag="p",
                                   name=f"p{qs}_{kt}")
                    p_of[kt] = p_sb
                    for j in range(2):
                        if j == 1:
                            pump((qs, idx, 4))
                        ps = psS.tile([128, 2, QS], f32, tag="ps",
                                      name=f"pp{qs}_{kt}_{j}")
                        for hh in range(2):
                            po = 64 * hh
                            nc.tensor.matmul(
                                ps[:, hh, c0:c1],
                                kt_sb[po : po + 64, j,
                                      kt * 128 : (kt + 1) * 128],
                                qt_sb[po : po + 64, j,
                                      qs * QS + c0 : qs * QS + c1],
                                start=True,
                                stop=True,
                            )
                        nc.scalar.activation(
                            p_sb[:, 2 * j : 2 * j + 2, c0:c1],
                            ps[:, :, c0:c1],
                            EXP,
                            scale=0.125,
                        )
                    for jj in interior:
                        nc.vector.memset(
                            p_sb[:, :, jj * 128 : (jj + 1) * 128], 0.0
                        )
                    for jj in partial_js:
                        bi = int(bidx[kt, 4 * qs + jj])
                        nc.vector.tensor_mul(
                            p_sb[:, :, jj * 128 : (jj + 1) * 128],
                            p_sb[:, :, jj * 128 : (jj + 1) * 128],
                            bias_sb[:, bi, :, :],
                        )
                    if idx == 1:
                        # strip boundary: the first PV waits on the previous
                        # strip's normalize reads of pots (WAR); give the PE
                        # filler work ahead of it
                        pop_one()
                        pop_one()
                        emit_pv(pots, p_of, c0_of, kts, idx - 1)
                    elif idx >= 2:
                        emit_pv(pots, p_of, c0_of, kts, idx - 1)
                        pop_one()
                        if qs >= 2:
                            # late strips are ACT-bound: drain the y-proj
                            # backlog into the PE slack instead of a tail
                            pop_one()
                pump((qs, len(kts), 2))
                emit_pv(pots, p_of, c0_of, kts, len(kts) - 1)
                # ---- normalize: invD = Exp(-Ln(D)) on ACT (same table set
                # as the exp stream, so no table reloads)
                invd = dn.tile([128, HG, QS], f32, tag="dn", name=f"dn{qs}")
                nc.scalar.activation(
                    invd[64:128, :, :], pots[64:128, :, :], LN
                )
                nc.scalar.activation(
                    invd[64:128, :, :], invd[64:128, :, :], EXP, scale=-1.0
                )
                for h in range(HG):
                    po = 64 * (h % 2)
                    nc.vector.tensor_mul(
                        ot_sb[po : po + 64, h // 2, qs * QS : (qs + 1) * QS],
                        pots[0:DK, h, :],
                        invd[64:128, h, :],
                    )
                for sti in range(QS // 128):
                    pend_y.append(qs * (QS // 128) + sti)
            while tasks or pend_y:
                pop_one()

    _split_waits(nc)
    return nc


_program_cache = {}


def get_program(cls, bidx, n_bias):
    key = (cls.tobytes(), bidx.tobytes(), n_bias)
    if key not in _program_cache:
        _program_cache[key] = build_program(cls, bidx, n_bias)
    return _program_cache[key]


def make_in_maps(q, k, v, mask, w_q, w_k, w_v, w_o, biases):
    if biases:
        # replicate each partial mask block across the four head slots of
        # the p tile so one DVE tensor_mul covers all heads
        bias_arr = np.stack(
            [np.repeat(b[:, None, :], HG, axis=1) for b in biases]
        ).astype(ml_dtypes.bfloat16)
    else:
        bias_arr = np.zeros((1, 128, HG, 128), ml_dtypes.bfloat16)
    def stage(mT, inner):
        # [R, C] -> [128, R//128, C] so each partition's DMA row is the
        # fully contiguous [R//128, C] block
        r, c = mT.shape
        return np.ascontiguousarray(
            mT.reshape(r // 128, 128, c).transpose(1, 0, 2)
        ).astype(ml_dtypes.bfloat16)

    in_maps = []
    for c in range(NCORES):
        b, g = divmod(c, 4)
        rows = slice(g * DHG, (g + 1) * DHG)
        in_maps.append(
            {
                "xqT": stage(q[b].T, S),
                "xkT": stage(k[b].T, S),
                "xvT": stage(v[b].T, S),
                "wqT": stage(w_q[rows].T, DHG),
                "wkT": stage(w_k[rows].T, DHG),
                "wvT": stage(w_v[rows].T, DHG),
                "woT": stage(w_o[:, rows].T, D),
                "biasT": bias_arr,
            }
        )
    return in_maps


def combine_results(results):
    out = np.empty((B, S, D), np.float32)
    for b in range(B):
        acc = results[4 * b]["y"].astype(np.float32)
        for g in range(1, 4):
            acc = acc + results[4 * b + g]["y"].astype(np.float32)
        out[b] = acc
    return out


def kernel(q, k, v, mask, w_q, w_k, w_v, w_o):
    q = np.asarray(q, np.float32)
    k = np.asarray(k, np.float32)
    v = np.asarray(v, np.float32)
    w_q = np.asarray(w_q, np.float32)
    w_k = np.asarray(w_k, np.float32)
    w_v = np.asarray(w_v, np.float32)
    w_o = np.asarray(w_o, np.float32)
    maskT = np.ascontiguousarray(
        np.broadcast_to(np.asarray(mask), (1, 1, S, S))[0, 0].T
    )
    cls, bidx, biases = classify_mask(maskT)
    nc = get_program(cls, bidx, len(biases))
    in_maps = make_in_maps(q, k, v, mask, w_q, w_k, w_v, w_o, biases)
    res = run_bass_kernel_spmd(nc, in_maps, list(range(NCORES)))
    return combine_results(res.results)


# revision 45
# speedup vs baseline: 1.1019x; 1.0498x over previous
"""Multi-head attention (B=2, S=2048, D=1024, H=16) on 8 trn2 NeuronCores.

Sharding: batch (2) x head-groups (4 heads each, 4 groups) = 8 cores.
Each core computes Q/K/V projections for its 4 heads on its batch,
causal-masked softmax attention, and a partial output projection
(row-sharded w_o); the host sums the 4 partials per batch.

Layout strategy: the host stages transposed inputs (xT = x[b].T) so every
matmul contraction runs over the SBUF partition axis with no on-device
transposes. Attention scores are computed transposed (ST[k, q]) so that
P = exp(ST) serves directly as the PV matmul's moving operand.

v3 structure (269us -> 207us -> this):
- Scores run un-padded K=64 with two heads packed concurrently in the PE
  array via base_partition 0/64 (tile_position row groups).
- The PV stationary operand is [V_h | 64 columns of ones]: output rows
  0:64 are the attention output, rows 64:128 all accumulate the softmax
  denominator D (broadcast by the matmul for free). The normalizer
  1/D = Exp(-Ln(D)) runs on the Activation engine (both functions live in
  the natural_log_exp_and_others table - one load, no switches).
- There is no separate projection phase: Q/K/V projection matmul groups
  are "filler" tasks with emission deadlines, woven into the attention
  strip loop wherever the (ACT-bound) attention chain leaves PE slack,
  alongside the previous strip's output-projection tiles. Scores for
  k-tile N+1 are issued before PV of k-tile N so the PE FIFO never
  head-of-line blocks on exp. The PE therefore stays continuously busy
  and the HAM clock gate stays at 2.4 GHz.
- x^T input DMAs are issued on the gpsimd (software DGE) queue, strip-
  interleaved q0,k0,v0,q1,... so the first projection group can start
  ~5us in; weight DMAs ride the sync queue; y output DMAs share gpsimd.
- PSUM: tag-shared ring of 2x 4KiB slots (score pair tiles [128,2,512]f32,
  y-projection tiles, and 2KiB projection tiles all rotate through it)
  + pots [128,4,512]f32 (4 banks) = 8 banks exactly.
"""
import sys

sys.path.insert(0, "/opt/trn_rl_repo")

import numpy as np
import ml_dtypes

import concourse.bass as bass
import concourse.mybir as mybir
import concourse.tile as tile
from concourse.bass_utils import run_bass_kernel_spmd

B, S, D, H, DK = 2, 2048, 1024, 16, 64
NCORES = 8
HG = 4                # heads per core
DHG = HG * DK         # 256 head-dims per core
KT = D // 128         # 8 contraction tiles for the projections
ST128 = S // 128      # 16 128-row tiles of S
QS = 512              # q-strip width
NQS = S // QS         # 4 strips

f32 = mybir.dt.float32
bf16 = mybir.dt.bfloat16
EXP = mybir.ActivationFunctionType.Exp
LN = mybir.ActivationFunctionType.Ln


def _split_waits(nc, max_waits=1):
    """This walrus build rejects >1 SyncWait per instruction (and >0 on
    fp32-family matmuls, which lower through the 1-wait S3_LW struct).
    Hoist excess waits onto dedicated NOPs on the same engine queue."""
    n = 0
    for fn in nc.m.functions:
        for blk in fn.blocks:
            new = []
            for ins in blk.instructions:
                si = getattr(ins, "sync_info", None)
                if si is not None and si.on_wait:
                    limit = 0 if isinstance(ins, mybir.InstMatmult) else max_waits
                    if len(si.on_wait) > limit:
                        waits = list(si.on_wait)
                        hoist = waits if limit == 0 else waits[:-limit]
                        keep = [] if limit == 0 else waits[-limit:]
                        for w in hoist:
                            n += 1
                            new.append(
                                mybir.InstNoOp(
                                    name=f"I-waitfix-{n}",
                                    engine=ins.engine,
                                    bass_nofuse=True,
                                    sync_info=mybir.SyncInfo(
                                        on_wait=[w], on_update=[]
                                    ),
                                )
                            )
                        ins.sync_info = mybir.SyncInfo(
                            on_wait=keep, on_update=list(si.on_update)
                        )
                new.append(ins)
            blk.instructions[:] = new
    return n


def classify_mask(maskT):
    """Block-classify the transposed mask at 128x128 granularity.
    Returns (cls[i,j] in {0 empty,1 full,2 partial}, bias index map,
    list of multiplicative fp32 mask blocks for the partial ones)."""
    nb = S // 128
    cls = np.empty((nb, nb), dtype=np.int8)
    bidx = np.full((nb, nb), -1, dtype=np.int32)
    biases = []
    seen = {}
    for i in range(nb):
        for j in range(nb):
            blk = maskT[i * 128 : (i + 1) * 128, j * 128 : (j + 1) * 128]
            if (blk != 0).all():
                cls[i, j] = 1
            elif (blk == 0).all():
                cls[i, j] = 0
            else:
                cls[i, j] = 2
                key = blk.tobytes()
                if key not in seen:
                    seen[key] = len(biases)
                    biases.append((blk != 0).astype(np.float32))
                bidx[i, j] = seen[key]
    return cls, bidx, biases


def strip_kts(cls, qs):
    sub_all = cls[:, 4 * qs : 4 * qs + 4]
    return [i for i in range(ST128) if sub_all[i].any()]


def build_program(cls, bidx, n_bias):
    nb_alloc = max(1, n_bias)
    nc = bass.Bass("TRN2", target_bir_lowering=False, debug=False,
                   num_devices=NCORES)
    # x^T and w^T are host-staged into the SBUF tile layouts so every DMA
    # row is >=3KiB contiguous (1KiB packets cap a queue at ~120GB/s)
    xq_d = nc.dram_tensor("xqT", [128, KT, S], bf16, kind="ExternalInput").ap()
    xk_d = nc.dram_tensor("xkT", [128, KT, S], bf16, kind="ExternalInput").ap()
    xv_d = nc.dram_tensor("xvT", [128, KT, S], bf16, kind="ExternalInput").ap()
    wq_d = nc.dram_tensor("wqT", [128, KT, DHG], bf16,
                          kind="ExternalInput").ap()
    wk_d = nc.dram_tensor("wkT", [128, KT, DHG], bf16,
                          kind="ExternalInput").ap()
    wv_d = nc.dram_tensor("wvT", [128, KT, DHG], bf16,
                          kind="ExternalInput").ap()
    wo_d = nc.dram_tensor("woT", [128, 2, D], bf16, kind="ExternalInput").ap()
    bias_d = nc.dram_tensor("biasT", [nb_alloc, 128, HG, 128], bf16,
                            kind="ExternalInput").ap()
    y_d = nc.dram_tensor("y", [S, D], bf16, kind="ExternalOutput").ap()

    with tile.TileContext(nc) as tc:
        with tc.tile_pool(name="persist", bufs=1) as pp, tc.tile_pool(
            name="xw", bufs=3
        ) as xw, tc.tile_pool(
            name="pb", bufs=4
        ) as pb, tc.tile_pool(
            name="dn", bufs=1
        ) as dn, tc.tile_pool(
            name="yb", bufs=2
        ) as ypool, tc.tile_pool(
            name="psS", bufs=2, space="PSUM"
        ) as psS, tc.tile_pool(
            name="psO", bufs=1, space="PSUM"
        ) as psO:
            # pair-major head layout: partitions 0:64 <-> head 2j, 64:128
            # <-> head 2j+1, for pair index j in {0,1}
            qt_sb = pp.tile([128, 2, S], bf16)            # Q^T
            kt_sb = pp.tile([128, 2, S], bf16)            # K^T
            v_sb = pp.tile([128, ST128, HG, 128], bf16)   # [V | ones]
            ot_sb = pp.tile([128, 2, S], bf16)            # attn out^T
            wo_sb = pp.tile([128, 2, D], bf16)
            bias_sb = pp.tile([128, nb_alloc, HG, 128], bf16)
            # denominator-broadcast columns of the PV stationary operand
            # (DVE is idle this early; gpsimd's queue is needed for DMAs)
            nc.vector.memset(v_sb[:, :, :, DK:], 1.0)

            # ---- input DMAs ----
            # Arrival order is engineered to match the fused schedule's
            # demand order: strip-0 x slices land first (gpsimd queue),
            # weights almost immediately (sync, now 4KiB rows), then
            # xq_rest (sync) for strip 1's Q projection, xk_rest and
            # xv_rest (gpsimd) for the later k-tiles. The scalar queue
            # stays clean for the exp stream.
            xts = {}
            wts = {}
            for which, x_d, w_d in (
                ("q", xq_d, wq_d), ("k", xk_d, wk_d), ("v", xv_d, wv_d)
            ):
                wt = xw.tile([128, KT, DHG], bf16, tag="wT",
                             name=f"wt{which}")
                nc.sync.dma_start(out=wt[:], in_=w_d)
                xts[which] = xw.tile([128, KT, S], bf16, tag="xT",
                                     name=f"xt{which}")
                wts[which] = wt
            for which, x_d in (("q", xq_d), ("k", xk_d), ("v", xv_d)):
                nc.gpsimd.dma_start(
                    out=xts[which][:, :, 0:QS], in_=x_d[:, :, 0:QS]
                )
            nc.sync.dma_start(out=wo_sb[:], in_=wo_d)
            if n_bias:
                nc.sync.dma_start(
                    out=bias_sb[:],
                    in_=bias_d.rearrange("n p a c -> p n a c"),
                )
            nc.sync.dma_start(
                out=xts["q"][:, :, QS:], in_=xq_d[:, :, QS:]
            )
            nc.gpsimd.dma_start(
                out=xts["k"][:, :, QS:], in_=xk_d[:, :, QS:]
            )
            nc.gpsimd.dma_start(
                out=xts["v"][:, :, QS:], in_=xv_d[:, :, QS:]
            )

            # ---- filler task machinery ----
            def qk_group(which, s, mt):
                def emit():
                    dst = qt_sb if which == "q" else kt_sb
                    ps = psS.tile([128, QS], f32, tag="ps",
                                  name=f"pj{which}{s}{mt}")
                    for kt in range(KT):
                        nc.tensor.matmul(
                            ps[:],
                            wts[which][:, kt, mt * 128 : (mt + 1) * 128],
                            xts[which][:, kt, s * QS : (s + 1) * QS],
                            start=(kt == 0),
                            stop=(kt == KT - 1),
                        )
                    nc.vector.tensor_copy(
                        out=dst[:, mt, s * QS : (s + 1) * QS], in_=ps[:]
                    )
                return emit

            def v_group(st):
                def emit():
                    ps = psS.tile([128, QS], f32, tag="ps", name=f"pjv{st}")
                    for kt in range(KT):
                        nc.tensor.matmul(
                            ps[:, :DHG],
                            xts["v"][:, kt, st * 128 : (st + 1) * 128],
                            wts["v"][:, kt, :],
                            start=(kt == 0),
                            stop=(kt == KT - 1),
                        )
                    nc.vector.tensor_copy(
                        out=v_sb[:, st, :, 0:DK],
                        in_=ps[:, :DHG].rearrange("p (h d) -> p h d", h=HG),
                    )
                return emit

            # deadlines: Q_s before (s,0); K_s before the strip's first
            # k-tile inside K-strip s; V_st before PV of kt=st in the first
            # strip whose k-range covers it (PV(idx) emits in slot idx+1)
            kts_of = {qs: strip_kts(cls, qs) for qs in range(NQS)}
            tasks = []
            for s in range(NQS):
                kts = kts_of[s]
                # strip 0: pair-0 weights land before the pair-0 scores of
                # the first k-tile so the exp stream starts ~5us earlier
                mt1_phase = 3 if s == 0 else 1
                tasks.append(((s, 0, 0), qk_group("q", s, 0)))
                tasks.append(((s, 0, mt1_phase), qk_group("q", s, 1)))
                kidx = next((i for i, kt in enumerate(kts)
                             if kt * 128 >= s * QS), 0)
                tasks.append(((s, kidx, 0), qk_group("k", s, 0)))
                tasks.append(((s, kidx, mt1_phase if kidx == 0 else 1),
                              qk_group("k", s, 1)))
            first_use = {}
            for s in range(NQS):
                for i, kt in enumerate(kts_of[s]):
                    if kt not in first_use:
                        first_use[kt] = (s, i + 1, 0)
            for st in range(ST128):
                if st in first_use:
                    tasks.append((first_use[st], v_group(st)))
            tasks.sort(key=lambda t: t[0])

            def pump(now):
                while tasks and tasks[0][0] <= now:
                    tasks.pop(0)[1]()

            def emit_yproj(st):
                yps = psS.tile([128, 2, QS], f32, tag="ps", name=f"yps{st}")
                for nh in range(2):
                    for mt in range(2):
                        nc.tensor.matmul(
                            yps[:, nh, :],
                            ot_sb[:, mt, st * 128 : (st + 1) * 128],
                            wo_sb[:, mt, nh * QS : (nh + 1) * QS],
                            start=(mt == 0),
                            stop=(mt == 1),
                        )
                y_sb = ypool.tile([128, 2 * QS], bf16, tag="y",
                                  name=f"ysb{st}")
                nc.vector.tensor_copy(
                    out=y_sb[:], in_=yps[:].rearrange("p a b -> p (a b)")
                )
                nc.gpsimd.dma_start(
                    out=y_d[st * 128 : (st + 1) * 128, :], in_=y_sb[:]
                )

            pend_y = []

            def pop_one(qs=NQS):
                # y-projections first: they depend only on completed strips
                # (never on a pending input DMA), so they can never stall
                # the PE FIFO. Projection groups are popped at most one
                # strip ahead of their deadline so a group whose x strip is
                # still in flight can't block the PE queue.
                if pend_y:
                    emit_yproj(pend_y.pop(0))
                elif tasks and tasks[0][0][0] <= qs + 1:
                    tasks.pop(0)[1]()

            def emit_pv(pots, p_of, c0_of, kts, idx):
                kt = kts[idx]
                c0 = c0_of[kt]
                if idx == 0 and c0 > 0:
                    nc.vector.memset(pots[:, :, 0:c0], 0.0)
                for h in range(HG):
                    nc.tensor.matmul(
                        pots[:, h, c0:],
                        v_sb[:, kt, h, :],
                        p_of[kt][:, h, c0:],
                        start=(idx == 0),
                        stop=(idx == len(kts) - 1),
                    )

            # ---- fused attention + projection strip loop ----
            for qs in range(NQS):
                sub_all = cls[:, 4 * qs : 4 * qs + 4]
                kts = kts_of[qs]
                pots = psO.tile([128, HG, QS], f32, tag="pot",
                                name=f"pot{qs}")
                c0_of = {}
                p_of = {}
                for idx, kt in enumerate(kts):
                    pump((qs, idx, 2))
                    sub = sub_all[kt]
                    nz = np.nonzero(sub)[0]
                    c0 = int(nz.min()) * 128
                    c1 = (int(nz.max()) + 1) * 128
                    c0_of[kt] = c0
                    partial_js = [j for j in range(4) if sub[j] == 2]
                    interior = [
                        j for j in range(4)
                        if sub[j] == 0 and c0 // 128 < j < c1 // 128
                    ]
                    # scores: per pair j one 2-bank psum tile; the pair's
                    # heads (row groups 0:64 / 64:128) run concurrently in
                    # disjoint PE subarrays
                    p_sb = pb.tile([128, HG, QS], bf16, tag="p",
                                   name=f"p{qs}_{kt}")
                    p_of[kt] = p_sb
                    for j in range(2):
                        if j == 1:
                            pump((qs, idx, 4))
                        ps = psS.tile([128, 2, QS], f32, tag="ps",
                                      name=f"pp{qs}_{kt}_{j}")
                        for hh in range(2):
                            po = 64 * hh
                            nc.tensor.matmul(
                                ps[:, hh, c0:c1],
                                kt_sb[po : po + 64, j,
                                      kt * 128 : (kt + 1) * 128],
                                qt_sb[po : po + 64, j,
                                      qs * QS + c0 : qs * QS + c1],
                                start=True,
                                stop=True,
                            )
                        nc.scalar.activation(
                            p_sb[:, 2 * j : 2 * j + 2, c0:c1],
                            ps[:, :, c0:c1],
                            EXP,
                            scale=0.125,
                        )
                    for jj in interior:
                        nc.vector.memset(
                            p_sb[:, :, jj * 128 : (jj + 1) * 128], 0.0
                        )
                    for jj in partial_js:
                        bi = int(bidx[kt, 4 * qs + jj])
                        nc.vector.tensor_mul(
                            p_sb[:, :, jj * 128 : (jj + 1) * 128],
                            p_sb[:, :, jj * 128 : (jj + 1) * 128],
                            bias_sb[:, bi, :, :],
                        )
                    if idx == 1:
                        # strip boundary: the first PV waits on the previous
                        # strip's normalize reads of pots (WAR); give the PE
                        # filler work ahead of it
                        pop_one(qs)
                        pop_one(qs)
                        emit_pv(pots, p_of, c0_of, kts, idx - 1)
                    elif idx >= 2:
                        emit_pv(pots, p_of, c0_of, kts, idx - 1)
                        pop_one(qs)
                        if qs >= 2:
                            # late strips are ACT-bound: drain the y-proj
                            # backlog into the PE slack instead of a tail
                            pop_one(qs)
                pump((qs, len(kts), 2))
                emit_pv(pots, p_of, c0_of, kts, len(kts) - 1)
                # ---- normalize: invD = Exp(-Ln(D)) on ACT (same table set
                # as the exp stream, so no table reloads)
                invd = dn.tile([128, HG, QS], f32, tag="dn", name=f"dn{qs}")
                nc.scalar.activation(
                    invd[64:128, :, :], pots[64:128, :, :], LN
                )
                nc.scalar.activation(
                    invd[64:128, :, :], invd[64:128, :, :], EXP, scale=-1.0
                )
                for h in range(HG):
                    po = 64 * (h % 2)
                    nc.vector.tensor_mul(
                        ot_sb[po : po + 64, h // 2, qs * QS : (qs + 1) * QS],
                        pots[0:DK, h, :],
                        invd[64:128, h, :],
                    )
                for sti in range(QS // 128):
                    pend_y.append(qs * (QS // 128) + sti)
            while tasks or pend_y:
                pop_one()

    _split_waits(nc)
    return nc


_program_cache = {}


def get_program(cls, bidx, n_bias):
    key = (cls.tobytes(), bidx.tobytes(), n_bias)
    if key not in _program_cache:
        _program_cache[key] = build_program(cls, bidx, n_bias)
    return _program_cache[key]


def make_in_maps(q, k, v, mask, w_q, w_k, w_v, w_o, biases):
    if biases:
        # replicate each partial mask block across the four head slots of
        # the p tile so one DVE tensor_mul covers all heads
        bias_arr = np.stack(
            [np.repeat(b[:, None, :], HG, axis=1) for b in biases]
        ).astype(ml_dtypes.bfloat16)
    else:
        bias_arr = np.zeros((1, 128, HG, 128), ml_dtypes.bfloat16)
    def stage(mT, inner):
        # [R, C] -> [128, R//128, C] so each partition's DMA row is the
        # fully contiguous [R//128, C] block
        r, c = mT.shape
        return np.ascontiguousarray(
            mT.reshape(r // 128, 128, c).transpose(1, 0, 2)
        ).astype(ml_dtypes.bfloat16)

    in_maps = []
    for c in range(NCORES):
        b, g = divmod(c, 4)
        rows = slice(g * DHG, (g + 1) * DHG)
        in_maps.append(
            {
                "xqT": stage(q[b].T, S),
                "xkT": stage(k[b].T, S),
                "xvT": stage(v[b].T, S),
                "wqT": stage(w_q[rows].T, DHG),
                "wkT": stage(w_k[rows].T, DHG),
                "wvT": stage(w_v[rows].T, DHG),
                "woT": stage(w_o[:, rows].T, D),
                "biasT": bias_arr,
            }
        )
    return in_maps


def combine_results(results):
    out = np.empty((B, S, D), np.float32)
    for b in range(B):
        acc = results[4 * b]["y"].astype(np.float32)
        for g in range(1, 4):
            acc = acc + results[4 * b + g]["y"].astype(np.float32)
        out[b] = acc
    return out


def kernel(q, k, v, mask, w_q, w_k, w_v, w_o):
    q = np.asarray(q, np.float32)
    k = np.asarray(k, np.float32)
    v = np.asarray(v, np.float32)
    w_q = np.asarray(w_q, np.float32)
    w_k = np.asarray(w_k, np.float32)
    w_v = np.asarray(w_v, np.float32)
    w_o = np.asarray(w_o, np.float32)
    maskT = np.ascontiguousarray(
        np.broadcast_to(np.asarray(mask), (1, 1, S, S))[0, 0].T
    )
    cls, bidx, biases = classify_mask(maskT)
    nc = get_program(cls, bidx, len(biases))
    in_maps = make_in_maps(q, k, v, mask, w_q, w_k, w_v, w_o, biases)
    res = run_bass_kernel_spmd(nc, in_maps, list(range(NCORES)))
    return combine_results(res.results)
